# revision 28
# baseline (speedup 1.0000x reference)
"""Trainium2 Bass kernel for nn_MultiHeadCulturalAttention.

Sharding (8 cores, SPMD single program with a partition-id branch):
  cores 0-3: "regular" branch — (batch b = core//2), 3 heads of hd=128 each
  cores 4-7: "cultural" branch — (batch b = (core-4)//2), 1 head of hd=384

Dataflow (per core, 384 projection columns):
  Q^T/K^T weight-stationary projections (f32r, transposed layout),
  interleaved with the attention pipeline so the scalar engine's Exp
  stream starts ~10us in (projection PSUM shares the psc/po pools).
  V evacuated into V_aug tiles [128, 3*129] bf16: per 128-col chunk the
  V columns are row-scaled by exp(attn_mask[s]) (folds the additive
  attention mask through the exp) and a scaled-ones column is appended
  (its AV output IS the softmax denominator). Scores^T per (s-pair,
  512-col t-window) into PSUM, bias-free Exp on the scalar engine with
  the softmax scale folded in. The cultural branch multiplies post-exp
  by host-precomputed ecm = exp(cmask^T + attn) in bf16.
  AV runs wt-STATIONARY (wt bf16 [128s,128t] x V_aug bf16 [128s,129]),
  producing [t, d]-layout PSUM where the denominator is per-partition:
  normalize = vector reciprocal + per-partition tensor_scalar. PE
  transposes flip o_n to [d, t] for the folded output projection
  wfold = branch_out_w @ out_w_half (f32r), interleaved per window.
  Host sums 4 partials per batch and adds a constant bias vector.
"""
import numpy as np
import ml_dtypes

import concourse.bass as bass
import concourse.mybir as mybir
from concourse import bacc
from concourse.tile import TileContext
from concourse.bass_utils import run_bass_kernel_spmd

F32 = mybir.dt.float32
F32R = mybir.dt.float32r
BF16 = mybir.dt.bfloat16
AF = mybir.ActivationFunctionType
ALU = mybir.AluOpType

B, T, E = 2, 2048, 768
NE = E // 128             # 6 e-chunks
NT = T // 128             # 16 tiles along seq (s-blocks / t-blocks)
F = 384                   # per-core projection width
NF = F // 128             # 3 f-tiles (heads for reg, d-chunks for cul)
NTW = 4                   # t-windows of 512
TW = 512
SCALE_REG = float(128 ** -0.5)
SCALE_CUL = float(384 ** -0.5)

_NC_CACHE = None


class _P:
    """Per-tag ring view of a shared pool (fixed bufs)."""
    def __init__(self, pool, bufs):
        self.pool, self.bufs = pool, bufs

    def tile(self, shape, dtype, tag="", name=None, **kw):
        return self.pool.tile(shape, dtype, tag=tag, name=name or tag or "t",
                              bufs=self.bufs, **kw)


def _proj_qk_1024(nc, pools, pool_out, sb_x, sb_w, sb_bias, f, tag):
    """One f-tile of Q^T/K^T via two [128,1024] psum tiles (psc ring)."""
    sb_o = pool_out.tile([128, T], BF16, tag=f"{tag}{f}", name=f"{tag}{f}")
    for th in range(2):
        ps = pools["psc"].tile([128, 1024], F32, tag="psc")
        for e in range(NE):
            for tq in range(2):
                nc.tensor.matmul(
                    ps[:, tq * 512:(tq + 1) * 512],
                    lhsT=sb_w[:, e * F + f * 128: e * F + (f + 1) * 128],
                    rhs=sb_x[:, e * T + th * 1024 + tq * 512: e * T + th * 1024 + (tq + 1) * 512],
                    start=(e == 0), stop=(e == NE - 1))
        nc.vector.tensor_scalar_add(
            sb_o[:, th * 1024:(th + 1) * 1024], ps[:], sb_bias[:, f:f + 1])
    return sb_o


def _proj_qk_512(nc, pools, pool_out, sb_x, sb_w, sb_bias, f, tag):
    """One f-tile of Q^T/K^T via four [128,512] psum tiles (psc ring)."""
    sb_o = pool_out.tile([128, T], BF16, tag=f"{tag}{f}", name=f"{tag}{f}")
    for tq in range(4):
        ps = pools["psc"].tile([128, TW], F32, tag="psc")
        for e in range(NE):
            nc.tensor.matmul(
                ps[:],
                lhsT=sb_w[:, e * F + f * 128: e * F + (f + 1) * 128],
                rhs=sb_x[:, e * T + tq * 512: e * T + (tq + 1) * 512],
                start=(e == 0), stop=(e == NE - 1))
        nc.vector.tensor_scalar_add(
            sb_o[:, tq * 512:(tq + 1) * 512], ps[:], sb_bias[:, f:f + 1])
    return sb_o


def _proj_v_aug(nc, pools, pool_out, sb_x, sb_wv, sb_easc, po_width):
    """V_aug[s]: [128, 3*129] bf16 via the po psum ring. Chunk c cols
    [c*129, c*129+128] = V[:, c-chunk] * easc[s] (row scale); col
    c*129+128 = easc[s]."""
    outs = []
    for s in range(NT):
        ps = pools["po"].tile([128, po_width], F32, tag="po")
        for e in range(NE):
            nc.tensor.matmul(
                ps[:, 0:F], lhsT=sb_x[:, e * T + s * 128: e * T + (s + 1) * 128],
                rhs=sb_wv[:, e * F:(e + 1) * F],
                start=(e == 0), stop=(e == NE - 1))
        va = pool_out.tile([128, NF * 129], BF16, tag=f"v{s}", name=f"v{s}")
        for c in range(NF):
            nc.vector.tensor_scalar_mul(
                va[:, c * 129: c * 129 + 128], ps[:, c * 128:(c + 1) * 128],
                sb_easc[:, s:s + 1])
            nc.vector.tensor_copy(
                va[:, c * 129 + 128: c * 129 + 129], sb_easc[:, s:s + 1])
        outs.append(va)
    return outs


def _se_phase_reg(nc, pools, sb_q, sb_k, m, tw, scale):
    """Scores + Exp for (map m, window tw), s in pairs. Returns 8 wt tiles
    [128, 1024] bf16 (= wt(2p) | wt(2p+1))."""
    wts = []
    for p in range(8):
        psc = pools["psc"].tile([128, 1024], F32, tag="psc")
        for half in range(2):
            s = 2 * p + half
            nc.tensor.matmul(
                psc[:, half * 512:(half + 1) * 512],
                lhsT=sb_k[m][:, s * 128:(s + 1) * 128],
                rhs=sb_q[m][:, tw * TW:(tw + 1) * TW],
                start=True, stop=True)
        wt = pools["wt"].tile([128, 1024], BF16, tag=f"wt{p}", name=f"wt{p}")
        nc.scalar.activation(wt[:], psc[:], AF.Exp, bias=0.0, scale=scale)
        wts.append(wt)
    return wts


def _se_phase_cul(nc, pools, sb_q, sb_k, tw, scale, d_ecm):
    """Scores (3-chunk accum) + Exp + ecm multiply, per s. Returns 16 wt
    tiles [128, 512] bf16."""
    wts = []
    for s in range(NT):
        psc = pools["psc"].tile([128, TW], F32, tag="psc")
        for c in range(NF):
            nc.tensor.matmul(
                psc[:],
                lhsT=sb_k[c][:, s * 128:(s + 1) * 128],
                rhs=sb_q[c][:, tw * TW:(tw + 1) * TW],
                start=(c == 0), stop=(c == NF - 1))
        ecm = pools["ecm"].tile([128, TW], BF16, tag=f"ecm{s % 4}")
        nc.sync.dma_start(
            out=ecm[:], in_=d_ecm[s * 128:(s + 1) * 128, tw * TW:(tw + 1) * TW])
        wr = pools["wtr"].tile([128, TW], BF16, tag=f"wtr{s % 3}")
        nc.scalar.activation(wr[:], psc[:], AF.Exp, bias=0.0, scale=scale)
        wt = pools["wt"].tile([128, TW], BF16, tag=f"wt{s}", name=f"cwt{s}")
        nc.vector.tensor_tensor(wt[:], wr[:], ecm[:], ALU.mult)
        wts.append(wt)
    return wts


def _av_reg_items(nc, pools, wts, sb_v, sb_ident, m, tw, fold_ctx=None):
    """Generator: AV + evac for regular (map m, window tw) in 4 items (one
    tb chunk each), so the caller can interleave score emission between
    them (avoids head-of-line blocking on the in-order tensor queue).
    wts: 8 x [128,1024] bf16. po [128,512] = two 129-wide slots."""
    if fold_ctx is not None:
        pf1 = pools["pf"].tile([128, 512], F32, tag="pf")
        fslots = [pf1[:, 0:256], pf1[:, 256:512]]
        fg = 0
    ptr = pools["ptr"].tile([128, 512], BF16, tag="ptr")
    otT = pools["outT"].tile([128, TW], BF16, tag=f"oT{m}", name=f"oT{m}")
    for tbp in range(2):
        po = pools["po"].tile([128, 512], F32, tag="po")
        for tb_i in range(2):
            tb = 2 * tbp + tb_i
            sl = po[:, tb_i * 256: tb_i * 256 + 129]
            first = True
            for p in range(8):
                for half in range(2):
                    s = 2 * p + half
                    nc.tensor.matmul(
                        sl,
                        lhsT=wts[p][:, half * 512 + tb * 128: half * 512 + (tb + 1) * 128],
                        rhs=sb_v[s][:, m * 129:(m + 1) * 129],
                        start=first, stop=(s == NT - 1))
                    first = False
            if tb_i == 0:
                yield otT
        for tb_i in range(2):
            tb = 2 * tbp + tb_i
            sl0 = tb_i * 256
            rec = pools["rec"].tile([128, 1], F32, tag=f"rec{tb % 2}")
            nc.vector.reciprocal(rec[:], po[:, sl0 + 128: sl0 + 129])
            o_n = pools["on"].tile([128, 128], BF16, tag=f"on{tb % 2}", name=f"on{tb%2}")
            nc.vector.tensor_scalar_mul(o_n[:], po[:, sl0: sl0 + 128], rec[:])
            nc.tensor.transpose(
                ptr[:, tb * 128:(tb + 1) * 128], o_n[:], sb_ident[:])
            nc.vector.tensor_copy(
                otT[:, tb * 128:(tb + 1) * 128], ptr[:, tb * 128:(tb + 1) * 128])
        if fold_ctx is not None:
            outT01, sb_wfold, d_out = fold_ctx
            for tt in (2 * tbp, 2 * tbp + 1):
                fg = _fold_tt(nc, pools, outT01 + [otT], sb_wfold, d_out,
                              tw, tt, fslots, fg)
        yield otT


def _av_reg(nc, pools, wts, sb_v, sb_ident, m, tw, fold_ctx=None):
    otT = None
    for otT in _av_reg_items(nc, pools, wts, sb_v, sb_ident, m, tw, fold_ctx):
        pass
    return otT


def _av_cul(nc, pools, wts, sb_v, sb_ident, tw, fold_ctx=None):
    """AV + evac for cultural window tw. wts: 16 x [128,512] bf16.
    po tile [128,387]: 3 chunk-groups of 129. fold_ctx (final window):
    (sb_wfold, d_out) — fold tt right after tb=tt's evacs."""
    if fold_ctx is not None:
        pf1 = pools["pf"].tile([128, 512], F32, tag="pf")
        fslots = [pf1[:, 0:256], pf1[:, 256:512]]
        fg = 0
    ptr = pools["ptr"].tile([128, 512], BF16, tag="ptr")
    otTs = [pools["outT"].tile([128, TW], BF16, tag=f"oT{c}", name=f"coT{c}")
            for c in range(NF)]
    for tb in range(4):
        po = pools["po"].tile([128, NF * 129], F32, tag="po")
        for c in range(NF):
            for s in range(NT):
                nc.tensor.matmul(
                    po[:, c * 129:(c + 1) * 129],
                    lhsT=wts[s][:, tb * 128:(tb + 1) * 128],
                    rhs=sb_v[s][:, c * 129:(c + 1) * 129],
                    start=(s == 0), stop=(s == NT - 1))
        rec = pools["rec"].tile([128, 1], F32, tag=f"rec{tb % 2}")
        nc.vector.reciprocal(rec[:], po[:, 128:129])
        for c in range(NF):
            o_n = pools["on"].tile([128, 128], BF16, tag=f"on{c % 2}", name=f"con{c%2}")
            nc.vector.tensor_scalar_mul(o_n[:], po[:, c * 129: c * 129 + 128], rec[:])
            nc.tensor.transpose(
                ptr[:, c * 128:(c + 1) * 128], o_n[:], sb_ident[:])
            nc.vector.tensor_copy(
                otTs[c][:, tb * 128:(tb + 1) * 128], ptr[:, c * 128:(c + 1) * 128])
        if fold_ctx is not None:
            sb_wfold, d_out = fold_ctx
            fg = _fold_tt(nc, pools, otTs, sb_wfold, d_out, tw, tb, fslots, fg)
    return otTs


def _fold_tt(nc, pools, outT_tiles, sb_wfold, d_out, tw, tt, slots, g0):
    """Fold one 128-row block tt of window tw using the given psum slots."""
    g = g0
    fin = pools["fin"].tile([128, E], F32, tag=f"fin{tt % 2}", name=f"fin{tt%2}")
    for eq in range(3):
        sl = slots[g % len(slots)]
        g += 1
        for c in range(NF):
            nc.tensor.matmul(
                sl,
                lhsT=outT_tiles[c][:, tt * 128:(tt + 1) * 128],
                rhs=sb_wfold[:, c * E + eq * 256: c * E + (eq + 1) * 256],
                start=(c == 0), stop=(c == NF - 1))
        nc.vector.tensor_copy(fin[:, eq * 256:(eq + 1) * 256], sl)
    nc.sync.dma_start(
        out=d_out[(tw * 4 + tt) * 128:(tw * 4 + tt + 1) * 128, :], in_=fin[:])
    return g


def _fold2(nc, pools, outT_tiles, sb_wfold, d_out, tw):
    """Fold window tw: 4 tt blocks x 3 e-quarter groups of 256 cols,
    ping-ponging over 4 psum slots (pf + ptr banks)."""
    pf1 = pools["pf"].tile([128, 512], F32, tag="pf")
    pf2 = pools["ptr"].tile([128, 512], F32, tag="ptr", name="pf2")
    slots = [pf1[:, 0:256], pf1[:, 256:512], pf2[:, 0:256], pf2[:, 256:512]]
    g = 0
    for tt in range(4):
        g = _fold_tt(nc, pools, outT_tiles, sb_wfold, d_out, tw, tt, slots, g)


def _branch_regular(nc, tc, pools, sb_x, sb_wq, sb_wk, sb_wv, sb_qb, sb_kb,
                    sb_easc, sb_ident, sb_wfold, d_out):
    sb_q, sb_k = [None] * NF, [None] * NF
    sb_v = None
    otT_by_tw = {}
    pending = None

    def se_with_av(m, tw, final=False):
        """Emit SE(m, tw) score-pairs interleaved with the pending AV's
        4 tb chunks (one chunk after every second pair)."""
        nonlocal pending
        gen = None
        if pending is not None:
            ptw, pm, pwts = pending
            if final:
                outT01 = otT_by_tw.pop(ptw)[:NF - 1]
                gen = _av_reg_items(nc, pools, pwts, sb_v, sb_ident, pm, ptw,
                                    fold_ctx=(outT01, sb_wfold, d_out))
            else:
                gen = _av_reg_items(nc, pools, pwts, sb_v, sb_ident, pm, ptw)
        wts = []
        otT = None
        for p in range(8):
            psc = pools["psc"].tile([128, 1024], F32, tag="psc")
            for half in range(2):
                s = 2 * p + half
                nc.tensor.matmul(
                    psc[:, half * 512:(half + 1) * 512],
                    lhsT=sb_k[m][:, s * 128:(s + 1) * 128],
                    rhs=sb_q[m][:, tw * TW:(tw + 1) * TW],
                    start=True, stop=True)
            wt = pools["wt"].tile([128, 1024], BF16, tag=f"wt{p}", name=f"wt{p}")
            nc.scalar.activation(wt[:], psc[:], AF.Exp, bias=0.0, scale=SCALE_REG)
            wts.append(wt)
            if gen is not None and p % 2 == 1:
                otT = next(gen, otT)
        if gen is not None:
            for otT in gen:
                pass
        if pending is not None and not final:
            ptw, pm, _ = pending
            otT_by_tw.setdefault(ptw, [None] * NF)[pm] = otT
            if pm == NF - 1:
                _fold2(nc, pools, otT_by_tw.pop(ptw), sb_wfold, d_out, ptw)
        pending = (tw, m, wts)

    def flush_final():
        nonlocal pending
        ptw, pm, pwts = pending
        outT01 = otT_by_tw.pop(ptw)[:NF - 1]
        _av_reg(nc, pools, pwts, sb_v, sb_ident, pm, ptw,
                fold_ctx=(outT01, sb_wfold, d_out))
        pending = None

    # interleaved prologue: projections feed the SE pipeline ASAP
    sb_k[0] = _proj_qk_1024(nc, pools, pools["kt"], sb_x, sb_wk, sb_kb, 0, "k")
    sb_q[0] = _proj_qk_1024(nc, pools, pools["qt"], sb_x, sb_wq, sb_qb, 0, "q")
    se_with_av(0, 0)
    sb_v = _proj_v_aug(nc, pools, pools["vp"], sb_x, sb_wv, sb_easc, 512)
    sb_k[1] = _proj_qk_1024(nc, pools, pools["kt"], sb_x, sb_wk, sb_kb, 1, "k")
    sb_q[1] = _proj_qk_1024(nc, pools, pools["qt"], sb_x, sb_wq, sb_qb, 1, "q")
    se_with_av(1, 0)
    sb_k[2] = _proj_qk_1024(nc, pools, pools["kt"], sb_x, sb_wk, sb_kb, 2, "k")
    sb_q[2] = _proj_qk_1024(nc, pools, pools["qt"], sb_x, sb_wq, sb_qb, 2, "q")
    # steady state
    steps = [(tw, m) for tw in range(NTW) for m in range(NF)]
    for tw, m in steps[2:]:
        se_with_av(m, tw)
    flush_final()


def _branch_cultural(nc, tc, pools, sb_x, sb_wq, sb_wk, sb_wv, sb_qb, sb_kb,
                     sb_easc, sb_ident, sb_wfold, d_out, d_ecm):
    sb_q, sb_k = [None] * NF, [None] * NF
    for c in range(NF):
        sb_k[c] = _proj_qk_512(nc, pools, pools["kt"], sb_x, sb_wk, sb_kb, c, "k")
        sb_q[c] = _proj_qk_512(nc, pools, pools["qt"], sb_x, sb_wq, sb_qb, c, "q")
    wts = _se_phase_cul(nc, pools, sb_q, sb_k, 0, SCALE_CUL, d_ecm)
    sb_v = _proj_v_aug(nc, pools, pools["vp"], sb_x, sb_wv, sb_easc, NF * 129)
    pending = (0, wts)
    for tw in range(1, NTW):
        wts = _se_phase_cul(nc, pools, sb_q, sb_k, tw, SCALE_CUL, d_ecm)
        ptw, pwts = pending
        otTs = _av_cul(nc, pools, pwts, sb_v, sb_ident, ptw)
        _fold2(nc, pools, otTs, sb_wfold, d_out, ptw)
        pending = (tw, wts)
    ptw, pwts = pending
    otTs = _av_cul(nc, pools, pwts, sb_v, sb_ident, ptw)
    _fold2(nc, pools, otTs, sb_wfold, d_out, ptw)


def _build_nc():
    nc = bacc.Bacc()
    d_x = nc.declare_dram_parameter("xall", [128, NE * T], BF16, isOutput=False)
    d_wq = nc.declare_dram_parameter("wq", [128, NE * F], BF16, isOutput=False)
    d_wk = nc.declare_dram_parameter("wk", [128, NE * F], BF16, isOutput=False)
    d_wv = nc.declare_dram_parameter("wv", [128, NE * F], BF16, isOutput=False)
    d_qb = nc.declare_dram_parameter("qb", [128, NF], F32, isOutput=False)
    d_kb = nc.declare_dram_parameter("kb", [128, NF], F32, isOutput=False)
    d_easc = nc.declare_dram_parameter("easc", [128, NT], F32, isOutput=False)
    d_wfold = nc.declare_dram_parameter("wfold", [128, NF * E], BF16, isOutput=False)
    d_ident = nc.declare_dram_parameter("ident", [128, 128], BF16, isOutput=False)
    d_ecm = nc.declare_dram_parameter("ecm", [T, T], BF16, isOutput=False)
    d_out = nc.declare_dram_parameter("out", [T, E], F32, isOutput=True)

    with TileContext(nc) as tc:
        pid = nc.partition_id()
        from contextlib import ExitStack
        with ExitStack() as stk:
            p_small = stk.enter_context(tc.tile_pool(name="small", bufs=1))
            p_xw = stk.enter_context(tc.tile_pool(name="xw", bufs=1))
            p_wfold = stk.enter_context(tc.tile_pool(name="wfp", bufs=1))

            sb_qb = p_small.tile([128, NF], F32)
            sb_kb = p_small.tile([128, NF], F32)
            sb_easc = p_small.tile([128, NT], F32)
            sb_ident = p_small.tile([128, 128], BF16)
            sb_wfold = p_wfold.tile([128, NF * E], BF16)

            # one big x DMA on the sync queue (single issue, splits
            # across all DMA engines); weights + small tensors on scalar
            sb_wq = p_xw.tile([128, NE * F], BF16)
            sb_wk = p_xw.tile([128, NE * F], BF16)
            sb_wv = p_xw.tile([128, NE * F], BF16)
            sb_x = p_xw.tile([128, NE * T], BF16, name="xall")
            nc.sync.dma_start(out=sb_x[:], in_=d_x[:])
            nc.scalar.dma_start(out=sb_wk[:], in_=d_wk[:])
            nc.scalar.dma_start(out=sb_wq[:], in_=d_wq[:])
            nc.scalar.dma_start(out=sb_wv[:], in_=d_wv[:])
            nc.scalar.dma_start(out=sb_qb[:], in_=d_qb[:])
            nc.scalar.dma_start(out=sb_kb[:], in_=d_kb[:])
            nc.scalar.dma_start(out=sb_easc[:], in_=d_easc[:])
            nc.scalar.dma_start(out=sb_ident[:], in_=d_ident[:])
            nc.scalar.dma_start(out=sb_wfold[:], in_=d_wfold[:])

            with tc.If(pid < 4) as cmp:
                with ExitStack() as astk:
                    pools = {
                        "psc": astk.enter_context(
                            tc.tile_pool(name="psc", bufs=2, space="PSUM")),
                        "po": astk.enter_context(
                            tc.tile_pool(name="po", bufs=2, space="PSUM")),
                        "ptr": astk.enter_context(
                            tc.tile_pool(name="ptr", bufs=1, space="PSUM")),
                        "pf": astk.enter_context(
                            tc.tile_pool(name="pf", bufs=1, space="PSUM")),
                        "wt": astk.enter_context(tc.tile_pool(name="wt", bufs=2)),
                        "rec": astk.enter_context(tc.tile_pool(name="rec", bufs=2)),
                        "on": astk.enter_context(tc.tile_pool(name="on", bufs=2)),
                        "outT": astk.enter_context(tc.tile_pool(name="outT", bufs=2)),
                        "fin": astk.enter_context(tc.tile_pool(name="fin", bufs=2)),
                        "qt": astk.enter_context(tc.tile_pool(name="qt", bufs=1)),
                        "kt": astk.enter_context(tc.tile_pool(name="kt", bufs=1)),
                        "vp": astk.enter_context(tc.tile_pool(name="vp", bufs=1)),
                    }
                    _branch_regular(nc, tc, pools, sb_x, sb_wq, sb_wk, sb_wv,
                                    sb_qb, sb_kb, sb_easc, sb_ident, sb_wfold,
                                    d_out)
            with cmp.Else():
                with ExitStack() as astk:
                    pools = {
                        "psc": astk.enter_context(
                            tc.tile_pool(name="cpsc", bufs=3, space="PSUM")),
                        "po": astk.enter_context(
                            tc.tile_pool(name="cpo", bufs=2, space="PSUM")),
                        "ptr": astk.enter_context(
                            tc.tile_pool(name="cptr", bufs=1, space="PSUM")),
                        "pf": astk.enter_context(
                            tc.tile_pool(name="cpf", bufs=1, space="PSUM")),
                        "wt": astk.enter_context(tc.tile_pool(name="cwt", bufs=2)),
                        "wtr": astk.enter_context(tc.tile_pool(name="cwtr", bufs=2)),
                        "ecm": astk.enter_context(tc.tile_pool(name="cecm", bufs=2)),
                        "rec": astk.enter_context(tc.tile_pool(name="crec", bufs=2)),
                        "on": astk.enter_context(tc.tile_pool(name="con", bufs=2)),
                        "outT": astk.enter_context(tc.tile_pool(name="coutT", bufs=2)),
                        "fin": astk.enter_context(tc.tile_pool(name="cfin", bufs=2)),
                        "qt": astk.enter_context(tc.tile_pool(name="cqt", bufs=1)),
                        "kt": astk.enter_context(tc.tile_pool(name="ckt", bufs=1)),
                        "vp": astk.enter_context(tc.tile_pool(name="cvp", bufs=1)),
                    }
                    _branch_cultural(nc, tc, pools, sb_x, sb_wq, sb_wk, sb_wv,
                                     sb_qb, sb_kb, sb_easc, sb_ident, sb_wfold,
                                     d_out, d_ecm)
    nc.compile()
    return nc


def _get_nc():
    global _NC_CACHE
    if _NC_CACHE is None:
        _NC_CACHE = _build_nc()
    return _NC_CACHE


def _chunked_T(a):
    """[in,out]-style [768, X] -> [128, 6*X] with e-chunk-major layout."""
    e, x = a.shape
    return np.ascontiguousarray(
        a.reshape(e // 128, 128, x).transpose(1, 0, 2).reshape(128, (e // 128) * x))


def kernel(hidden_states, cultural_mask, attention_mask,
           rq_w, rk_w, rv_w, ro_w, cq_w, ck_w, cv_w, co_w,
           rq_b, rk_b, rv_b, ro_b, cq_b, ck_b, cv_b, co_b,
           r_cb, c_cb, out_w, out_b):
    hidden_states = np.asarray(hidden_states)
    nc = _get_nc()
    Wo1 = np.asarray(out_w[:E], np.float64)
    Wo2 = np.asarray(out_w[E:], np.float64)
    wfold_reg = (np.asarray(ro_w, np.float64) @ Wo1)
    wfold_cul = (np.asarray(co_w, np.float64) @ Wo2)
    r_cb_flat = np.asarray(r_cb, np.float64).reshape(-1)
    c_cb_flat = np.asarray(c_cb, np.float64).reshape(-1)
    qb_reg_full = np.asarray(rq_b, np.float64) + r_cb_flat
    qb_cul_full = np.asarray(cq_b, np.float64) + c_cb_flat

    ident = np.eye(128).astype(ml_dtypes.bfloat16)
    ecm_zero = np.zeros((T, T), ml_dtypes.bfloat16)
    ones_easc = np.ones((128, NT), np.float32)
    in_maps = []
    for core in range(8):
        if core < 4:
            b, h0 = core // 2, (core % 2) * 3
            cols = slice(h0 * 128, h0 * 128 + F)
            wq_l, wk_l, wv_l = rq_w[:, cols], rk_w[:, cols], rv_w[:, cols]
            qb_l = qb_reg_full[cols]
            kb_l = np.asarray(rk_b, np.float64)[cols]
            wfold_l = wfold_reg[cols]
            ecm_l = ecm_zero
            easc_l = np.ascontiguousarray(
                np.exp(np.asarray(attention_mask[b, 0, 0, :], np.float64))
                .astype(np.float32).reshape(NT, 128).T)
        else:
            b, h = (core - 4) // 2, (core - 4) % 2
            cols = slice(h * F, (h + 1) * F)
            wq_l, wk_l, wv_l = cq_w[:, cols], ck_w[:, cols], cv_w[:, cols]
            qb_l = qb_cul_full[cols]
            kb_l = np.asarray(ck_b, np.float64)[cols]
            wfold_l = wfold_cul[cols]
            cm = (np.asarray(cultural_mask[b], np.float64).T
                  + np.asarray(attention_mask[b, 0, 0, :], np.float64)[:, None])
            ecm_l = np.exp(cm).astype(ml_dtypes.bfloat16)
            easc_l = ones_easc
        xT = np.ascontiguousarray(np.asarray(hidden_states[b], np.float32).T)
        im = {
            "wq": _chunked_T(np.asarray(wq_l, np.float32)).astype(ml_dtypes.bfloat16),
            "wk": _chunked_T(np.asarray(wk_l, np.float32)).astype(ml_dtypes.bfloat16),
            "wv": _chunked_T(np.asarray(wv_l, np.float32)).astype(ml_dtypes.bfloat16),
            "qb": np.ascontiguousarray(np.asarray(qb_l, np.float32).reshape(NF, 128).T),
            "kb": np.ascontiguousarray(np.asarray(kb_l, np.float32).reshape(NF, 128).T),
            "easc": easc_l,
            "wfold": _chunked_T(np.asarray(wfold_l, np.float32)).astype(ml_dtypes.bfloat16),
            "ident": ident,
            "ecm": ecm_l,
        }
        im["xall"] = _chunked_T(xT).astype(ml_dtypes.bfloat16)
        in_maps.append(im)

    res = run_bass_kernel_spmd(nc, in_maps, list(range(8))).results

    bias_total = (np.asarray(out_b, np.float64)
                  + np.asarray(ro_b, np.float64) @ Wo1
                  + np.asarray(co_b, np.float64) @ Wo2
                  + np.asarray(rv_b, np.float64) @ np.asarray(ro_w, np.float64) @ Wo1
                  + np.asarray(cv_b, np.float64) @ np.asarray(co_w, np.float64) @ Wo2)
    out = np.empty((B, T, E), np.float32)
    for b in range(B):
        acc = (res[2 * b]["out"].astype(np.float64)
               + res[2 * b + 1]["out"].astype(np.float64)
               + res[4 + 2 * b]["out"].astype(np.float64)
               + res[5 + 2 * b]["out"].astype(np.float64)
               + bias_total)
        out[b] = acc.astype(np.float32)
    return out


# revision 29
# speedup vs baseline: 1.0119x; 1.0119x over previous
"""Trainium2 Bass kernel for nn_MultiHeadCulturalAttention.

Sharding (8 cores, SPMD single program with a partition-id branch):
  cores 0-3: "regular" branch — (batch b = core//2), 3 heads of hd=128 each
  cores 4-7: "cultural" branch — (batch b = (core-4)//2), 1 head of hd=384

Dataflow (per core, 384 projection columns):
  Q^T/K^T weight-stationary projections (f32r, transposed layout),
  interleaved with the attention pipeline so the scalar engine's Exp
  stream starts ~10us in (projection PSUM shares the psc/po pools).
  V evacuated into V_aug tiles [128, 3*129] bf16: per 128-col chunk the
  V columns are row-scaled by exp(attn_mask[s]) (folds the additive
  attention mask through the exp) and a scaled-ones column is appended
  (its AV output IS the softmax denominator). Scores^T per (s-pair,
  512-col t-window) into PSUM, bias-free Exp on the scalar engine with
  the softmax scale folded in. The cultural branch multiplies post-exp
  by host-precomputed ecm = exp(cmask^T + attn) in bf16.
  AV runs wt-STATIONARY (wt bf16 [128s,128t] x V_aug bf16 [128s,129]),
  producing [t, d]-layout PSUM where the denominator is per-partition:
  normalize = vector reciprocal + per-partition tensor_scalar. PE
  transposes flip o_n to [d, t] for the folded output projection
  wfold = branch_out_w @ out_w_half (f32r), interleaved per window.
  Host sums 4 partials per batch and adds a constant bias vector.
"""
import numpy as np
import ml_dtypes

import concourse.bass as bass
import concourse.mybir as mybir
from concourse import bacc
from concourse.tile import TileContext
from concourse.bass_utils import run_bass_kernel_spmd

F32 = mybir.dt.float32
F32R = mybir.dt.float32r
BF16 = mybir.dt.bfloat16
AF = mybir.ActivationFunctionType
ALU = mybir.AluOpType

B, T, E = 2, 2048, 768
NE = E // 128             # 6 e-chunks
NT = T // 128             # 16 tiles along seq (s-blocks / t-blocks)
F = 384                   # per-core projection width
NF = F // 128             # 3 f-tiles (heads for reg, d-chunks for cul)
NTW = 4                   # t-windows of 512
TW = 512
SCALE_REG = float(128 ** -0.5)
SCALE_CUL = float(384 ** -0.5)

_NC_CACHE = None


class _P:
    """Per-tag ring view of a shared pool (fixed bufs)."""
    def __init__(self, pool, bufs):
        self.pool, self.bufs = pool, bufs

    def tile(self, shape, dtype, tag="", name=None, **kw):
        return self.pool.tile(shape, dtype, tag=tag, name=name or tag or "t",
                              bufs=self.bufs, **kw)


def _proj_qk_1024(nc, pools, pool_out, sb_x, sb_w, sb_bias, f, tag):
    """One f-tile of Q^T/K^T via two [128,1024] psum tiles (psc ring)."""
    sb_o = pool_out.tile([128, T], BF16, tag=f"{tag}{f}", name=f"{tag}{f}")
    for th in range(2):
        ps = pools["psc"].tile([128, 1024], F32, tag="psc")
        for e in range(NE):
            for tq in range(2):
                nc.tensor.matmul(
                    ps[:, tq * 512:(tq + 1) * 512],
                    lhsT=sb_w[:, e * F + f * 128: e * F + (f + 1) * 128],
                    rhs=sb_x[:, e * T + th * 1024 + tq * 512: e * T + th * 1024 + (tq + 1) * 512],
                    start=(e == 0), stop=(e == NE - 1))
        nc.vector.tensor_scalar_add(
            sb_o[:, th * 1024:(th + 1) * 1024], ps[:], sb_bias[:, f:f + 1])
    return sb_o


def _proj_qk_512(nc, pools, pool_out, sb_x, sb_w, sb_bias, f, tag):
    """One f-tile of Q^T/K^T via four [128,512] psum tiles (psc ring)."""
    sb_o = pool_out.tile([128, T], BF16, tag=f"{tag}{f}", name=f"{tag}{f}")
    for tq in range(4):
        ps = pools["psc"].tile([128, TW], F32, tag="psc")
        for e in range(NE):
            nc.tensor.matmul(
                ps[:],
                lhsT=sb_w[:, e * F + f * 128: e * F + (f + 1) * 128],
                rhs=sb_x[:, e * T + tq * 512: e * T + (tq + 1) * 512],
                start=(e == 0), stop=(e == NE - 1))
        nc.vector.tensor_scalar_add(
            sb_o[:, tq * 512:(tq + 1) * 512], ps[:], sb_bias[:, f:f + 1])
    return sb_o


def _proj_v_aug(nc, pools, pool_out, sb_x, sb_wv, sb_easc, po_width):
    """V_aug[s]: [128, 3*129] bf16 via the po psum ring. Chunk c cols
    [c*129, c*129+128] = V[:, c-chunk] * easc[s] (row scale); col
    c*129+128 = easc[s]."""
    outs = []
    for s in range(NT):
        ps = pools["po"].tile([128, po_width], F32, tag="po")
        for e in range(NE):
            nc.tensor.matmul(
                ps[:, 0:F], lhsT=sb_x[:, e * T + s * 128: e * T + (s + 1) * 128],
                rhs=sb_wv[:, e * F:(e + 1) * F],
                start=(e == 0), stop=(e == NE - 1))
        va = pool_out.tile([128, NF * 129], BF16, tag=f"v{s}", name=f"v{s}")
        for c in range(NF):
            nc.vector.tensor_scalar_mul(
                va[:, c * 129: c * 129 + 128], ps[:, c * 128:(c + 1) * 128],
                sb_easc[:, s:s + 1])
            nc.vector.tensor_copy(
                va[:, c * 129 + 128: c * 129 + 129], sb_easc[:, s:s + 1])
        outs.append(va)
    return outs


def _se_phase_reg(nc, pools, sb_q, sb_k, m, tw, scale):
    """Scores + Exp for (map m, window tw), s in pairs. Returns 8 wt tiles
    [128, 1024] bf16 (= wt(2p) | wt(2p+1))."""
    wts = []
    for p in range(8):
        psc = pools["psc"].tile([128, 1024], F32, tag="psc")
        for half in range(2):
            s = 2 * p + half
            nc.tensor.matmul(
                psc[:, half * 512:(half + 1) * 512],
                lhsT=sb_k[m][:, s * 128:(s + 1) * 128],
                rhs=sb_q[m][:, tw * TW:(tw + 1) * TW],
                start=True, stop=True)
        wt = pools["wt"].tile([128, 1024], BF16, tag=f"wt{p}", name=f"wt{p}")
        nc.scalar.activation(wt[:], psc[:], AF.Exp, bias=0.0, scale=scale)
        wts.append(wt)
    return wts


def _se_phase_cul(nc, pools, sb_q, sb_k, tw, scale, d_ecm):
    """Scores (3-chunk accum) + Exp + ecm multiply, per s. Returns 16 wt
    tiles [128, 512] bf16."""
    wts = []
    for s in range(NT):
        psc = pools["psc"].tile([128, TW], F32, tag="psc")
        for c in range(NF):
            nc.tensor.matmul(
                psc[:],
                lhsT=sb_k[c][:, s * 128:(s + 1) * 128],
                rhs=sb_q[c][:, tw * TW:(tw + 1) * TW],
                start=(c == 0), stop=(c == NF - 1))
        ecm = pools["ecm"].tile([128, TW], BF16, tag=f"ecm{s % 4}")
        nc.sync.dma_start(
            out=ecm[:], in_=d_ecm[s * 128:(s + 1) * 128, tw * TW:(tw + 1) * TW])
        wr = pools["wtr"].tile([128, TW], BF16, tag=f"wtr{s % 3}")
        nc.scalar.activation(wr[:], psc[:], AF.Exp, bias=0.0, scale=scale)
        wt = pools["wt"].tile([128, TW], BF16, tag=f"wt{s}", name=f"cwt{s}")
        nc.vector.tensor_tensor(wt[:], wr[:], ecm[:], ALU.mult)
        wts.append(wt)
    return wts


def _av_reg(nc, pools, wts, sb_v, sb_ident, m, tw, fold_ctx=None):
    """AV + evac for regular (map m, window tw). wts: 8 x [128,1024] bf16.
    po [128,512] = two 129-wide accumulator slots (tb pairs). fold_ctx
    (final window): (outT01, sb_wfold, d_out) — fold tt pairs between
    AV tb pairs so the tail overlaps."""
    if fold_ctx is not None:
        pf1 = pools["pf"].tile([128, 512], F32, tag="pf")
        fslots = [pf1[:, 0:256], pf1[:, 256:512]]
        fg = 0
    ptr = pools["ptr"].tile([128, 512], BF16, tag="ptr")
    otT = pools["outT"].tile([128, TW], BF16, tag=f"oT{m}", name=f"oT{m}")
    for tbp in range(2):
        po = pools["po"].tile([128, 512], F32, tag="po")
        for tb_i in range(2):
            tb = 2 * tbp + tb_i
            sl = po[:, tb_i * 256: tb_i * 256 + 129]
            first = True
            for p in range(8):
                for half in range(2):
                    s = 2 * p + half
                    nc.tensor.matmul(
                        sl,
                        lhsT=wts[p][:, half * 512 + tb * 128: half * 512 + (tb + 1) * 128],
                        rhs=sb_v[s][:, m * 129:(m + 1) * 129],
                        start=first, stop=(s == NT - 1))
                    first = False
        for tb_i in range(2):
            tb = 2 * tbp + tb_i
            sl0 = tb_i * 256
            rec = pools["rec"].tile([128, 1], F32, tag=f"rec{tb % 2}")
            nc.vector.reciprocal(rec[:], po[:, sl0 + 128: sl0 + 129])
            o_n = pools["on"].tile([128, 128], BF16, tag=f"on{tb % 2}", name=f"on{tb%2}")
            nc.vector.tensor_scalar_mul(o_n[:], po[:, sl0: sl0 + 128], rec[:])
            nc.tensor.transpose(
                ptr[:, tb * 128:(tb + 1) * 128], o_n[:], sb_ident[:])
            nc.vector.tensor_copy(
                otT[:, tb * 128:(tb + 1) * 128], ptr[:, tb * 128:(tb + 1) * 128])
        if fold_ctx is not None:
            outT01, sb_wfold, d_out = fold_ctx
            for tt in (2 * tbp, 2 * tbp + 1):
                fg = _fold_tt(nc, pools, outT01 + [otT], sb_wfold, d_out,
                              tw, tt, fslots, fg)
    return otT


def _av_cul(nc, pools, wts, sb_v, sb_ident, tw, fold_ctx=None):
    """AV + evac for cultural window tw. wts: 16 x [128,512] bf16.
    po tile [128,387]: 3 chunk-groups of 129. fold_ctx (final window):
    (sb_wfold, d_out) — fold tt right after tb=tt's evacs."""
    if fold_ctx is not None:
        pf1 = pools["pf"].tile([128, 512], F32, tag="pf")
        fslots = [pf1[:, 0:256], pf1[:, 256:512]]
        fg = 0
    ptr = pools["ptr"].tile([128, 512], BF16, tag="ptr")
    otTs = [pools["outT"].tile([128, TW], BF16, tag=f"oT{c}", name=f"coT{c}")
            for c in range(NF)]
    for tb in range(4):
        po = pools["po"].tile([128, NF * 129], F32, tag="po")
        for c in range(NF):
            for s in range(NT):
                nc.tensor.matmul(
                    po[:, c * 129:(c + 1) * 129],
                    lhsT=wts[s][:, tb * 128:(tb + 1) * 128],
                    rhs=sb_v[s][:, c * 129:(c + 1) * 129],
                    start=(s == 0), stop=(s == NT - 1))
        rec = pools["rec"].tile([128, 1], F32, tag=f"rec{tb % 2}")
        nc.vector.reciprocal(rec[:], po[:, 128:129])
        for c in range(NF):
            o_n = pools["on"].tile([128, 128], BF16, tag=f"on{c % 2}", name=f"con{c%2}")
            nc.vector.tensor_scalar_mul(o_n[:], po[:, c * 129: c * 129 + 128], rec[:])
            nc.tensor.transpose(
                ptr[:, c * 128:(c + 1) * 128], o_n[:], sb_ident[:])
            nc.vector.tensor_copy(
                otTs[c][:, tb * 128:(tb + 1) * 128], ptr[:, c * 128:(c + 1) * 128])
        if fold_ctx is not None:
            sb_wfold, d_out = fold_ctx
            fg = _fold_tt(nc, pools, otTs, sb_wfold, d_out, tw, tb, fslots, fg)
    return otTs


def _fold_tt(nc, pools, outT_tiles, sb_wfold, d_out, tw, tt, slots, g0):
    """Fold one 128-row block tt of window tw using the given psum slots."""
    g = g0
    fin = pools["fin"].tile([128, E], F32, tag=f"fin{tt % 2}", name=f"fin{tt%2}")
    for eq in range(3):
        sl = slots[g % len(slots)]
        g += 1
        for c in range(NF):
            nc.tensor.matmul(
                sl,
                lhsT=outT_tiles[c][:, tt * 128:(tt + 1) * 128],
                rhs=sb_wfold[:, c * E + eq * 256: c * E + (eq + 1) * 256],
                start=(c == 0), stop=(c == NF - 1))
        nc.vector.tensor_copy(fin[:, eq * 256:(eq + 1) * 256], sl)
    nc.sync.dma_start(
        out=d_out[(tw * 4 + tt) * 128:(tw * 4 + tt + 1) * 128, :], in_=fin[:])
    return g


def _fold2(nc, pools, outT_tiles, sb_wfold, d_out, tw):
    """Fold window tw: 4 tt blocks x 3 e-quarter groups of 256 cols,
    ping-ponging over 4 psum slots (pf + ptr banks)."""
    pf1 = pools["pf"].tile([128, 512], F32, tag="pf")
    pf2 = pools["ptr"].tile([128, 512], F32, tag="ptr", name="pf2")
    slots = [pf1[:, 0:256], pf1[:, 256:512], pf2[:, 0:256], pf2[:, 256:512]]
    g = 0
    for tt in range(4):
        g = _fold_tt(nc, pools, outT_tiles, sb_wfold, d_out, tw, tt, slots, g)


def _branch_regular(nc, tc, pools, sb_x, sb_wq, sb_wk, sb_wv, sb_qb, sb_kb,
                    sb_easc, sb_ident, sb_wfold, d_out):
    sb_q, sb_k = [None] * NF, [None] * NF
    sb_v = None
    otT_by_tw = {}
    pending = None

    def flush(final=False):
        nonlocal pending
        if pending is None:
            return
        ptw, pm, pwts = pending
        otT = _av_reg(nc, pools, pwts, sb_v, sb_ident, pm, ptw)
        otT_by_tw.setdefault(ptw, [None] * NF)[pm] = otT
        if pm == NF - 1:
            _fold2(nc, pools, otT_by_tw.pop(ptw), sb_wfold, d_out, ptw)
        pending = None

    # interleaved prologue: projections feed the SE pipeline ASAP
    sb_k[0] = _proj_qk_1024(nc, pools, pools["kt"], sb_x, sb_wk, sb_kb, 0, "k")
    sb_q[0] = _proj_qk_1024(nc, pools, pools["qt"], sb_x, sb_wq, sb_qb, 0, "q")
    wts = _se_phase_reg(nc, pools, sb_q, sb_k, 0, 0, SCALE_REG)
    sb_v = _proj_v_aug(nc, pools, pools["vp"], sb_x, sb_wv, sb_easc, 512)
    nxt = (0, 0, wts)
    sb_k[1] = _proj_qk_1024(nc, pools, pools["kt"], sb_x, sb_wk, sb_kb, 1, "k")
    sb_q[1] = _proj_qk_1024(nc, pools, pools["qt"], sb_x, sb_wq, sb_qb, 1, "q")
    pending = nxt
    wts = _se_phase_reg(nc, pools, sb_q, sb_k, 1, 0, SCALE_REG)
    flush()
    nxt = (0, 1, wts)
    sb_k[2] = _proj_qk_1024(nc, pools, pools["kt"], sb_x, sb_wk, sb_kb, 2, "k")
    sb_q[2] = _proj_qk_1024(nc, pools, pools["qt"], sb_x, sb_wq, sb_qb, 2, "q")
    pending = nxt
    # steady state
    steps = [(tw, m) for tw in range(NTW) for m in range(NF)]
    for tw, m in steps[2:]:
        wts = _se_phase_reg(nc, pools, sb_q, sb_k, m, tw, SCALE_REG)
        flush()
        pending = (tw, m, wts)
    flush(final=True)


def _branch_cultural(nc, tc, pools, sb_x, sb_wq, sb_wk, sb_wv, sb_qb, sb_kb,
                     sb_easc, sb_ident, sb_wfold, d_out, d_ecm):
    sb_q, sb_k = [None] * NF, [None] * NF
    for c in range(NF):
        sb_k[c] = _proj_qk_512(nc, pools, pools["kt"], sb_x, sb_wk, sb_kb, c, "k")
        sb_q[c] = _proj_qk_512(nc, pools, pools["qt"], sb_x, sb_wq, sb_qb, c, "q")
    wts = _se_phase_cul(nc, pools, sb_q, sb_k, 0, SCALE_CUL, d_ecm)
    sb_v = _proj_v_aug(nc, pools, pools["vp"], sb_x, sb_wv, sb_easc, NF * 129)
    pending = (0, wts)
    for tw in range(1, NTW):
        wts = _se_phase_cul(nc, pools, sb_q, sb_k, tw, SCALE_CUL, d_ecm)
        ptw, pwts = pending
        otTs = _av_cul(nc, pools, pwts, sb_v, sb_ident, ptw)
        _fold2(nc, pools, otTs, sb_wfold, d_out, ptw)
        pending = (tw, wts)
    ptw, pwts = pending
    otTs = _av_cul(nc, pools, pwts, sb_v, sb_ident, ptw)
    _fold2(nc, pools, otTs, sb_wfold, d_out, ptw)


def _build_nc():
    nc = bacc.Bacc()
    d_x = nc.declare_dram_parameter("xall", [128, NE * T], BF16, isOutput=False)
    d_wq = nc.declare_dram_parameter("wq", [128, NE * F], BF16, isOutput=False)
    d_wk = nc.declare_dram_parameter("wk", [128, NE * F], BF16, isOutput=False)
    d_wv = nc.declare_dram_parameter("wv", [128, NE * F], BF16, isOutput=False)
    d_qb = nc.declare_dram_parameter("qb", [128, NF], F32, isOutput=False)
    d_kb = nc.declare_dram_parameter("kb", [128, NF], F32, isOutput=False)
    d_easc = nc.declare_dram_parameter("easc", [128, NT], F32, isOutput=False)
    d_wfold = nc.declare_dram_parameter("wfold", [128, NF * E], BF16, isOutput=False)
    d_ident = nc.declare_dram_parameter("ident", [128, 128], BF16, isOutput=False)
    d_ecm = nc.declare_dram_parameter("ecm", [T, T], BF16, isOutput=False)
    d_out = nc.declare_dram_parameter("out", [T, E], F32, isOutput=True)

    with TileContext(nc) as tc:
        pid = nc.partition_id()
        from contextlib import ExitStack
        with ExitStack() as stk:
            p_small = stk.enter_context(tc.tile_pool(name="small", bufs=1))
            p_xw = stk.enter_context(tc.tile_pool(name="xw", bufs=1))
            p_wfold = stk.enter_context(tc.tile_pool(name="wfp", bufs=1))

            sb_qb = p_small.tile([128, NF], F32)
            sb_kb = p_small.tile([128, NF], F32)
            sb_easc = p_small.tile([128, NT], F32)
            sb_ident = p_small.tile([128, 128], BF16)
            sb_wfold = p_wfold.tile([128, NF * E], BF16)

            # one big x DMA on the sync queue (single issue, splits
            # across all DMA engines); weights + small tensors on scalar
            sb_wq = p_xw.tile([128, NE * F], BF16)
            sb_wk = p_xw.tile([128, NE * F], BF16)
            sb_wv = p_xw.tile([128, NE * F], BF16)
            sb_x = p_xw.tile([128, NE * T], BF16, name="xall")
            nc.sync.dma_start(out=sb_x[:], in_=d_x[:])
            nc.scalar.dma_start(out=sb_wk[:], in_=d_wk[:])
            nc.scalar.dma_start(out=sb_wq[:], in_=d_wq[:])
            nc.scalar.dma_start(out=sb_wv[:], in_=d_wv[:])
            nc.scalar.dma_start(out=sb_qb[:], in_=d_qb[:])
            nc.scalar.dma_start(out=sb_kb[:], in_=d_kb[:])
            nc.scalar.dma_start(out=sb_easc[:], in_=d_easc[:])
            nc.scalar.dma_start(out=sb_ident[:], in_=d_ident[:])
            nc.scalar.dma_start(out=sb_wfold[:], in_=d_wfold[:])

            with tc.If(pid < 4) as cmp:
                with ExitStack() as astk:
                    pools = {
                        "psc": astk.enter_context(
                            tc.tile_pool(name="psc", bufs=2, space="PSUM")),
                        "po": astk.enter_context(
                            tc.tile_pool(name="po", bufs=2, space="PSUM")),
                        "ptr": astk.enter_context(
                            tc.tile_pool(name="ptr", bufs=1, space="PSUM")),
                        "pf": astk.enter_context(
                            tc.tile_pool(name="pf", bufs=1, space="PSUM")),
                        "wt": astk.enter_context(tc.tile_pool(name="wt", bufs=2)),
                        "rec": astk.enter_context(tc.tile_pool(name="rec", bufs=2)),
                        "on": astk.enter_context(tc.tile_pool(name="on", bufs=2)),
                        "outT": astk.enter_context(tc.tile_pool(name="outT", bufs=2)),
                        "fin": astk.enter_context(tc.tile_pool(name="fin", bufs=2)),
                        "qt": astk.enter_context(tc.tile_pool(name="qt", bufs=1)),
                        "kt": astk.enter_context(tc.tile_pool(name="kt", bufs=1)),
                        "vp": astk.enter_context(tc.tile_pool(name="vp", bufs=1)),
                    }
                    _branch_regular(nc, tc, pools, sb_x, sb_wq, sb_wk, sb_wv,
                                    sb_qb, sb_kb, sb_easc, sb_ident, sb_wfold,
                                    d_out)
            with cmp.Else():
                with ExitStack() as astk:
                    pools = {
                        "psc": astk.enter_context(
                            tc.tile_pool(name="cpsc", bufs=3, space="PSUM")),
                        "po": astk.enter_context(
                            tc.tile_pool(name="cpo", bufs=2, space="PSUM")),
                        "ptr": astk.enter_context(
                            tc.tile_pool(name="cptr", bufs=1, space="PSUM")),
                        "pf": astk.enter_context(
                            tc.tile_pool(name="cpf", bufs=1, space="PSUM")),
                        "wt": astk.enter_context(tc.tile_pool(name="cwt", bufs=2)),
                        "wtr": astk.enter_context(tc.tile_pool(name="cwtr", bufs=2)),
                        "ecm": astk.enter_context(tc.tile_pool(name="cecm", bufs=2)),
                        "rec": astk.enter_context(tc.tile_pool(name="crec", bufs=2)),
                        "on": astk.enter_context(tc.tile_pool(name="con", bufs=2)),
                        "outT": astk.enter_context(tc.tile_pool(name="coutT", bufs=2)),
                        "fin": astk.enter_context(tc.tile_pool(name="cfin", bufs=2)),
                        "qt": astk.enter_context(tc.tile_pool(name="cqt", bufs=1)),
                        "kt": astk.enter_context(tc.tile_pool(name="ckt", bufs=1)),
                        "vp": astk.enter_context(tc.tile_pool(name="cvp", bufs=1)),
                    }
                    _branch_cultural(nc, tc, pools, sb_x, sb_wq, sb_wk, sb_wv,
                                     sb_qb, sb_kb, sb_easc, sb_ident, sb_wfold,
                                     d_out, d_ecm)
    nc.compile()
    return nc


def _get_nc():
    global _NC_CACHE
    if _NC_CACHE is None:
        _NC_CACHE = _build_nc()
    return _NC_CACHE


def _chunked_T(a):
    """[in,out]-style [768, X] -> [128, 6*X] with e-chunk-major layout."""
    e, x = a.shape
    return np.ascontiguousarray(
        a.reshape(e // 128, 128, x).transpose(1, 0, 2).reshape(128, (e // 128) * x))


def kernel(hidden_states, cultural_mask, attention_mask,
           rq_w, rk_w, rv_w, ro_w, cq_w, ck_w, cv_w, co_w,
           rq_b, rk_b, rv_b, ro_b, cq_b, ck_b, cv_b, co_b,
           r_cb, c_cb, out_w, out_b):
    hidden_states = np.asarray(hidden_states)
    nc = _get_nc()
    Wo1 = np.asarray(out_w[:E], np.float64)
    Wo2 = np.asarray(out_w[E:], np.float64)
    wfold_reg = (np.asarray(ro_w, np.float64) @ Wo1)
    wfold_cul = (np.asarray(co_w, np.float64) @ Wo2)
    r_cb_flat = np.asarray(r_cb, np.float64).reshape(-1)
    c_cb_flat = np.asarray(c_cb, np.float64).reshape(-1)
    qb_reg_full = np.asarray(rq_b, np.float64) + r_cb_flat
    qb_cul_full = np.asarray(cq_b, np.float64) + c_cb_flat

    ident = np.eye(128).astype(ml_dtypes.bfloat16)
    ecm_zero = np.zeros((T, T), ml_dtypes.bfloat16)
    ones_easc = np.ones((128, NT), np.float32)
    in_maps = []
    for core in range(8):
        if core < 4:
            b, h0 = core // 2, (core % 2) * 3
            cols = slice(h0 * 128, h0 * 128 + F)
            wq_l, wk_l, wv_l = rq_w[:, cols], rk_w[:, cols], rv_w[:, cols]
            qb_l = qb_reg_full[cols]
            kb_l = np.asarray(rk_b, np.float64)[cols]
            wfold_l = wfold_reg[cols]
            ecm_l = ecm_zero
            easc_l = np.ascontiguousarray(
                np.exp(np.asarray(attention_mask[b, 0, 0, :], np.float64))
                .astype(np.float32).reshape(NT, 128).T)
        else:
            b, h = (core - 4) // 2, (core - 4) % 2
            cols = slice(h * F, (h + 1) * F)
            wq_l, wk_l, wv_l = cq_w[:, cols], ck_w[:, cols], cv_w[:, cols]
            qb_l = qb_cul_full[cols]
            kb_l = np.asarray(ck_b, np.float64)[cols]
            wfold_l = wfold_cul[cols]
            cm = (np.asarray(cultural_mask[b], np.float64).T
                  + np.asarray(attention_mask[b, 0, 0, :], np.float64)[:, None])
            ecm_l = np.exp(cm).astype(ml_dtypes.bfloat16)
            easc_l = ones_easc
        xT = np.ascontiguousarray(np.asarray(hidden_states[b], np.float32).T)
        im = {
            "wq": _chunked_T(np.asarray(wq_l, np.float32)).astype(ml_dtypes.bfloat16),
            "wk": _chunked_T(np.asarray(wk_l, np.float32)).astype(ml_dtypes.bfloat16),
            "wv": _chunked_T(np.asarray(wv_l, np.float32)).astype(ml_dtypes.bfloat16),
            "qb": np.ascontiguousarray(np.asarray(qb_l, np.float32).reshape(NF, 128).T),
            "kb": np.ascontiguousarray(np.asarray(kb_l, np.float32).reshape(NF, 128).T),
            "easc": easc_l,
            "wfold": _chunked_T(np.asarray(wfold_l, np.float32)).astype(ml_dtypes.bfloat16),
            "ident": ident,
            "ecm": ecm_l,
        }
        im["xall"] = _chunked_T(xT).astype(ml_dtypes.bfloat16)
        in_maps.append(im)

    res = run_bass_kernel_spmd(nc, in_maps, list(range(8))).results

    bias_total = (np.asarray(out_b, np.float64)
                  + np.asarray(ro_b, np.float64) @ Wo1
                  + np.asarray(co_b, np.float64) @ Wo2
                  + np.asarray(rv_b, np.float64) @ np.asarray(ro_w, np.float64) @ Wo1
                  + np.asarray(cv_b, np.float64) @ np.asarray(co_w, np.float64) @ Wo2)
    out = np.empty((B, T, E), np.float32)
    for b in range(B):
        acc = (res[2 * b]["out"].astype(np.float64)
               + res[2 * b + 1]["out"].astype(np.float64)
               + res[4 + 2 * b]["out"].astype(np.float64)
               + res[5 + 2 * b]["out"].astype(np.float64)
               + bias_total)
        out[b] = acc.astype(np.float32)
    return out


# revision 30
# speedup vs baseline: 1.0579x; 1.0455x over previous
"""Trainium2 Bass kernel for nn_MultiHeadCulturalAttention.

Sharding (8 cores, SPMD single program with a partition-id branch):
  cores 0-3: "regular" branch — (batch b = core//2), 3 heads of hd=128 each
  cores 4-7: "cultural" branch — (batch b = (core-4)//2), 1 head of hd=384

Dataflow (per core, 384 projection columns; x/weights/Q/K in bf16,
f32 PSUM accumulation):
  Q^T/K^T weight-stationary projections, interleaved with the
  attention pipeline (projection PSUM shares the psc/po pool rings;
  one large x DMA on the sync queue, weights on the scalar queue).
  V evacuated into V_aug tiles [128, 3*129] bf16: per 128-col chunk
  the V columns are row-scaled by exp(attn_mask[s]) (folds the
  additive attention mask through the exp) and a scaled-ones column
  is appended (its AV output IS the softmax denominator). Scores^T
  per (s-pair, 512-col t-window) into PSUM, bias-free [128,1024] Exp
  on the scalar engine with the softmax scale folded in. The cultural
  branch multiplies post-exp by host-precomputed
  ecm = exp(cmask^T + attn) in bf16 ([128,512] Exps, psc ring 3).
  AV runs wt-STATIONARY (wt bf16 [128s,128t] x V_aug bf16 [128s,129]),
  producing [t, d]-layout PSUM where the denominator is per-partition:
  normalize = vector reciprocal + per-partition tensor_scalar. bf16 PE
  transposes flip o_n to [d, t] for the folded output projection
  wfold = branch_out_w @ out_w_half (bf16), interleaved per window
  (fold psum ping-pongs 4 slots across the pf + ptr banks; the final
  window folds between AV tb pairs). All evacuations run on the
  vector engine; gpsimd is off the data path. Host sums 4 partials
  per batch and adds a constant bias vector.

PSUM (8 banks, both branches exactly full): psc 2x[128,1024] (reg) /
3x[128,512] (cul), po 2x1 bank, ptr 1, pf 1. Measured ~189-192us vs
347us baseline; rel err 5.1e-3 (gate 2e-2).
"""
import numpy as np
import ml_dtypes

import concourse.bass as bass
import concourse.mybir as mybir
from concourse import bacc
from concourse.tile import TileContext
from concourse.bass_utils import run_bass_kernel_spmd

F32 = mybir.dt.float32
F32R = mybir.dt.float32r
BF16 = mybir.dt.bfloat16
AF = mybir.ActivationFunctionType
ALU = mybir.AluOpType

B, T, E = 2, 2048, 768
NE = E // 128             # 6 e-chunks
NT = T // 128             # 16 tiles along seq (s-blocks / t-blocks)
F = 384                   # per-core projection width
NF = F // 128             # 3 f-tiles (heads for reg, d-chunks for cul)
NTW = 4                   # t-windows of 512
TW = 512
SCALE_REG = float(128 ** -0.5)
SCALE_CUL = float(384 ** -0.5)

_NC_CACHE = None


class _P:
    """Per-tag ring view of a shared pool (fixed bufs)."""
    def __init__(self, pool, bufs):
        self.pool, self.bufs = pool, bufs

    def tile(self, shape, dtype, tag="", name=None, **kw):
        return self.pool.tile(shape, dtype, tag=tag, name=name or tag or "t",
                              bufs=self.bufs, **kw)


def _proj_qk_1024(nc, pools, pool_out, sb_x, sb_w, sb_bias, f, tag):
    """One f-tile of Q^T/K^T via two [128,1024] psum tiles (psc ring)."""
    sb_o = pool_out.tile([128, T], BF16, tag=f"{tag}{f}", name=f"{tag}{f}")
    for th in range(2):
        ps = pools["psc"].tile([128, 1024], F32, tag="psc")
        for e in range(NE):
            for tq in range(2):
                nc.tensor.matmul(
                    ps[:, tq * 512:(tq + 1) * 512],
                    lhsT=sb_w[:, e * F + f * 128: e * F + (f + 1) * 128],
                    rhs=sb_x[:, e * T + th * 1024 + tq * 512: e * T + th * 1024 + (tq + 1) * 512],
                    start=(e == 0), stop=(e == NE - 1))
        nc.vector.tensor_scalar_add(
            sb_o[:, th * 1024:(th + 1) * 1024], ps[:], sb_bias[:, f:f + 1])
    return sb_o


def _proj_qk_512(nc, pools, pool_out, sb_x, sb_w, sb_bias, f, tag):
    """One f-tile of Q^T/K^T via four [128,512] psum tiles (psc ring)."""
    sb_o = pool_out.tile([128, T], BF16, tag=f"{tag}{f}", name=f"{tag}{f}")
    for tq in range(4):
        ps = pools["psc"].tile([128, TW], F32, tag="psc")
        for e in range(NE):
            nc.tensor.matmul(
                ps[:],
                lhsT=sb_w[:, e * F + f * 128: e * F + (f + 1) * 128],
                rhs=sb_x[:, e * T + tq * 512: e * T + (tq + 1) * 512],
                start=(e == 0), stop=(e == NE - 1))
        nc.vector.tensor_scalar_add(
            sb_o[:, tq * 512:(tq + 1) * 512], ps[:], sb_bias[:, f:f + 1])
    return sb_o


def _proj_v_aug(nc, pools, pool_out, sb_x, sb_wv, sb_easc, po_width):
    """V_aug[s]: [128, 3*129] bf16 via the po psum ring. Chunk c cols
    [c*129, c*129+128] = V[:, c-chunk] * easc[s] (row scale); col
    c*129+128 = easc[s]."""
    outs = []
    for s in range(NT):
        ps = pools["po"].tile([128, po_width], F32, tag="po")
        for e in range(NE):
            nc.tensor.matmul(
                ps[:, 0:F], lhsT=sb_x[:, e * T + s * 128: e * T + (s + 1) * 128],
                rhs=sb_wv[:, e * F:(e + 1) * F],
                start=(e == 0), stop=(e == NE - 1))
        va = pool_out.tile([128, NF * 129], BF16, tag=f"v{s}", name=f"v{s}")
        for c in range(NF):
            nc.vector.tensor_scalar_mul(
                va[:, c * 129: c * 129 + 128], ps[:, c * 128:(c + 1) * 128],
                sb_easc[:, s:s + 1])
            nc.vector.tensor_copy(
                va[:, c * 129 + 128: c * 129 + 129], sb_easc[:, s:s + 1])
        outs.append(va)
    return outs


def _se_phase_reg(nc, pools, sb_q, sb_k, m, tw, scale):
    """Scores + Exp for (map m, window tw), s in pairs. Returns 8 wt tiles
    [128, 1024] bf16 (= wt(2p) | wt(2p+1))."""
    wts = []
    for p in range(8):
        psc = pools["psc"].tile([128, 1024], F32, tag="psc")
        for half in range(2):
            s = 2 * p + half
            nc.tensor.matmul(
                psc[:, half * 512:(half + 1) * 512],
                lhsT=sb_k[m][:, s * 128:(s + 1) * 128],
                rhs=sb_q[m][:, tw * TW:(tw + 1) * TW],
                start=True, stop=True)
        wt = pools["wt"].tile([128, 1024], BF16, tag=f"wt{p}", name=f"wt{p}")
        nc.scalar.activation(wt[:], psc[:], AF.Exp, bias=0.0, scale=scale)
        wts.append(wt)
    return wts


def _se_phase_cul(nc, pools, sb_q, sb_k, tw, scale, d_ecm):
    """Scores (3-chunk accum) + Exp + ecm multiply, per s. Returns 16 wt
    tiles [128, 512] bf16."""
    wts = []
    for s in range(NT):
        psc = pools["psc"].tile([128, TW], F32, tag="psc")
        for c in range(NF):
            nc.tensor.matmul(
                psc[:],
                lhsT=sb_k[c][:, s * 128:(s + 1) * 128],
                rhs=sb_q[c][:, tw * TW:(tw + 1) * TW],
                start=(c == 0), stop=(c == NF - 1))
        ecm = pools["ecm"].tile([128, TW], BF16, tag=f"ecm{s % 4}")
        nc.sync.dma_start(
            out=ecm[:], in_=d_ecm[s * 128:(s + 1) * 128, tw * TW:(tw + 1) * TW])
        wr = pools["wtr"].tile([128, TW], BF16, tag=f"wtr{s % 3}")
        nc.scalar.activation(wr[:], psc[:], AF.Exp, bias=0.0, scale=scale)
        wt = pools["wt"].tile([128, TW], BF16, tag=f"wt{s}", name=f"cwt{s}")
        nc.vector.tensor_tensor(wt[:], wr[:], ecm[:], ALU.mult)
        wts.append(wt)
    return wts


def _av_reg(nc, pools, wts, sb_v, sb_ident, m, tw, fold_ctx=None):
    """AV + evac for regular (map m, window tw). wts: 8 x [128,1024] bf16.
    po [128,512] = two 129-wide accumulator slots (tb pairs). fold_ctx
    (final window): (outT01, sb_wfold, d_out) — fold tt pairs between
    AV tb pairs so the tail overlaps."""
    if fold_ctx is not None:
        pf1 = pools["pf"].tile([128, 512], F32, tag="pf")
        fslots = [pf1[:, 0:256], pf1[:, 256:512]]
        fg = 0
    ptr = pools["ptr"].tile([128, 512], BF16, tag="ptr")
    otT = pools["outT"].tile([128, TW], BF16, tag=f"oT{m}", name=f"oT{m}")
    for tbp in range(2):
        po = pools["po"].tile([128, 512], F32, tag="po")
        for tb_i in range(2):
            tb = 2 * tbp + tb_i
            sl = po[:, tb_i * 256: tb_i * 256 + 129]
            first = True
            for p in range(8):
                for half in range(2):
                    s = 2 * p + half
                    nc.tensor.matmul(
                        sl,
                        lhsT=wts[p][:, half * 512 + tb * 128: half * 512 + (tb + 1) * 128],
                        rhs=sb_v[s][:, m * 129:(m + 1) * 129],
                        start=first, stop=(s == NT - 1))
                    first = False
        for tb_i in range(2):
            tb = 2 * tbp + tb_i
            sl0 = tb_i * 256
            rec = pools["rec"].tile([128, 1], F32, tag=f"rec{tb % 2}")
            nc.vector.reciprocal(rec[:], po[:, sl0 + 128: sl0 + 129])
            o_n = pools["on"].tile([128, 128], BF16, tag=f"on{tb % 2}", name=f"on{tb%2}")
            nc.vector.tensor_scalar_mul(o_n[:], po[:, sl0: sl0 + 128], rec[:])
            nc.tensor.transpose(
                ptr[:, tb * 128:(tb + 1) * 128], o_n[:], sb_ident[:])
            nc.vector.tensor_copy(
                otT[:, tb * 128:(tb + 1) * 128], ptr[:, tb * 128:(tb + 1) * 128])
        if fold_ctx is not None:
            outT01, sb_wfold, d_out = fold_ctx
            for tt in (2 * tbp, 2 * tbp + 1):
                fg = _fold_tt(nc, pools, outT01 + [otT], sb_wfold, d_out,
                              tw, tt, fslots, fg)
    return otT


def _av_cul(nc, pools, wts, sb_v, sb_ident, tw, fold_ctx=None):
    """AV + evac for cultural window tw. wts: 16 x [128,512] bf16.
    po tile [128,387]: 3 chunk-groups of 129. fold_ctx (final window):
    (sb_wfold, d_out) — fold tt right after tb=tt's evacs."""
    if fold_ctx is not None:
        pf1 = pools["pf"].tile([128, 512], F32, tag="pf")
        fslots = [pf1[:, 0:256], pf1[:, 256:512]]
        fg = 0
    ptr = pools["ptr"].tile([128, 512], BF16, tag="ptr")
    otTs = [pools["outT"].tile([128, TW], BF16, tag=f"oT{c}", name=f"coT{c}")
            for c in range(NF)]
    for tb in range(4):
        po = pools["po"].tile([128, NF * 129], F32, tag="po")
        for c in range(NF):
            for s in range(NT):
                nc.tensor.matmul(
                    po[:, c * 129:(c + 1) * 129],
                    lhsT=wts[s][:, tb * 128:(tb + 1) * 128],
                    rhs=sb_v[s][:, c * 129:(c + 1) * 129],
                    start=(s == 0), stop=(s == NT - 1))
        rec = pools["rec"].tile([128, 1], F32, tag=f"rec{tb % 2}")
        nc.vector.reciprocal(rec[:], po[:, 128:129])
        for c in range(NF):
            o_n = pools["on"].tile([128, 128], BF16, tag=f"on{c % 2}", name=f"con{c%2}")
            nc.vector.tensor_scalar_mul(o_n[:], po[:, c * 129: c * 129 + 128], rec[:])
            nc.tensor.transpose(
                ptr[:, c * 128:(c + 1) * 128], o_n[:], sb_ident[:])
            nc.vector.tensor_copy(
                otTs[c][:, tb * 128:(tb + 1) * 128], ptr[:, c * 128:(c + 1) * 128])
        if fold_ctx is not None:
            sb_wfold, d_out = fold_ctx
            fg = _fold_tt(nc, pools, otTs, sb_wfold, d_out, tw, tb, fslots, fg)
    return otTs


def _fold_tt(nc, pools, outT_tiles, sb_wfold, d_out, tw, tt, slots, g0):
    """Fold one 128-row block tt of window tw using the given psum slots."""
    g = g0
    fin = pools["fin"].tile([128, E], F32, tag=f"fin{tt % 2}", name=f"fin{tt%2}")
    for eq in range(3):
        sl = slots[g % len(slots)]
        g += 1
        for c in range(NF):
            nc.tensor.matmul(
                sl,
                lhsT=outT_tiles[c][:, tt * 128:(tt + 1) * 128],
                rhs=sb_wfold[:, c * E + eq * 256: c * E + (eq + 1) * 256],
                start=(c == 0), stop=(c == NF - 1))
        nc.vector.tensor_copy(fin[:, eq * 256:(eq + 1) * 256], sl)
    nc.sync.dma_start(
        out=d_out[(tw * 4 + tt) * 128:(tw * 4 + tt + 1) * 128, :], in_=fin[:])
    return g


def _fold2(nc, pools, outT_tiles, sb_wfold, d_out, tw):
    """Fold window tw: 4 tt blocks x 3 e-quarter groups of 256 cols,
    ping-ponging over 4 psum slots (pf + ptr banks)."""
    pf1 = pools["pf"].tile([128, 512], F32, tag="pf")
    pf2 = pools["ptr"].tile([128, 512], F32, tag="ptr", name="pf2")
    slots = [pf1[:, 0:256], pf1[:, 256:512], pf2[:, 0:256], pf2[:, 256:512]]
    g = 0
    for tt in range(4):
        g = _fold_tt(nc, pools, outT_tiles, sb_wfold, d_out, tw, tt, slots, g)


def _branch_regular(nc, tc, pools, sb_x, sb_wq, sb_wk, sb_wv, sb_qb, sb_kb,
                    sb_easc, sb_ident, sb_wfold, d_out):
    sb_q, sb_k = [None] * NF, [None] * NF
    sb_v = None
    otT_by_tw = {}
    pending = None

    def flush(final=False):
        nonlocal pending
        if pending is None:
            return
        ptw, pm, pwts = pending
        otT = _av_reg(nc, pools, pwts, sb_v, sb_ident, pm, ptw)
        otT_by_tw.setdefault(ptw, [None] * NF)[pm] = otT
        if pm == NF - 1:
            _fold2(nc, pools, otT_by_tw.pop(ptw), sb_wfold, d_out, ptw)
        pending = None

    # interleaved prologue: projections feed the SE pipeline ASAP
    sb_k[0] = _proj_qk_1024(nc, pools, pools["kt"], sb_x, sb_wk, sb_kb, 0, "k")
    sb_q[0] = _proj_qk_1024(nc, pools, pools["qt"], sb_x, sb_wq, sb_qb, 0, "q")
    wts = _se_phase_reg(nc, pools, sb_q, sb_k, 0, 0, SCALE_REG)
    sb_v = _proj_v_aug(nc, pools, pools["vp"], sb_x, sb_wv, sb_easc, 512)
    nxt = (0, 0, wts)
    sb_k[1] = _proj_qk_1024(nc, pools, pools["kt"], sb_x, sb_wk, sb_kb, 1, "k")
    sb_q[1] = _proj_qk_1024(nc, pools, pools["qt"], sb_x, sb_wq, sb_qb, 1, "q")
    pending = nxt
    wts = _se_phase_reg(nc, pools, sb_q, sb_k, 1, 0, SCALE_REG)
    flush()
    nxt = (0, 1, wts)
    sb_k[2] = _proj_qk_1024(nc, pools, pools["kt"], sb_x, sb_wk, sb_kb, 2, "k")
    sb_q[2] = _proj_qk_1024(nc, pools, pools["qt"], sb_x, sb_wq, sb_qb, 2, "q")
    pending = nxt
    # steady state
    steps = [(tw, m) for tw in range(NTW) for m in range(NF)]
    for tw, m in steps[2:]:
        wts = _se_phase_reg(nc, pools, sb_q, sb_k, m, tw, SCALE_REG)
        flush()
        pending = (tw, m, wts)
    flush(final=True)


def _branch_cultural(nc, tc, pools, sb_x, sb_wq, sb_wk, sb_wv, sb_qb, sb_kb,
                     sb_easc, sb_ident, sb_wfold, d_out, d_ecm):
    sb_q, sb_k = [None] * NF, [None] * NF
    for c in range(NF):
        sb_k[c] = _proj_qk_512(nc, pools, pools["kt"], sb_x, sb_wk, sb_kb, c, "k")
        sb_q[c] = _proj_qk_512(nc, pools, pools["qt"], sb_x, sb_wq, sb_qb, c, "q")
    wts = _se_phase_cul(nc, pools, sb_q, sb_k, 0, SCALE_CUL, d_ecm)
    sb_v = _proj_v_aug(nc, pools, pools["vp"], sb_x, sb_wv, sb_easc, NF * 129)
    pending = (0, wts)
    for tw in range(1, NTW):
        wts = _se_phase_cul(nc, pools, sb_q, sb_k, tw, SCALE_CUL, d_ecm)
        ptw, pwts = pending
        otTs = _av_cul(nc, pools, pwts, sb_v, sb_ident, ptw)
        _fold2(nc, pools, otTs, sb_wfold, d_out, ptw)
        pending = (tw, wts)
    ptw, pwts = pending
    otTs = _av_cul(nc, pools, pwts, sb_v, sb_ident, ptw)
    _fold2(nc, pools, otTs, sb_wfold, d_out, ptw)


def _build_nc():
    nc = bacc.Bacc()
    d_x = nc.declare_dram_parameter("xall", [128, NE * T], BF16, isOutput=False)
    d_wq = nc.declare_dram_parameter("wq", [128, NE * F], BF16, isOutput=False)
    d_wk = nc.declare_dram_parameter("wk", [128, NE * F], BF16, isOutput=False)
    d_wv = nc.declare_dram_parameter("wv", [128, NE * F], BF16, isOutput=False)
    d_qb = nc.declare_dram_parameter("qb", [128, NF], F32, isOutput=False)
    d_kb = nc.declare_dram_parameter("kb", [128, NF], F32, isOutput=False)
    d_easc = nc.declare_dram_parameter("easc", [128, NT], F32, isOutput=False)
    d_wfold = nc.declare_dram_parameter("wfold", [128, NF * E], BF16, isOutput=False)
    d_ident = nc.declare_dram_parameter("ident", [128, 128], BF16, isOutput=False)
    d_ecm = nc.declare_dram_parameter("ecm", [T, T], BF16, isOutput=False)
    d_out = nc.declare_dram_parameter("out", [T, E], F32, isOutput=True)

    with TileContext(nc) as tc:
        pid = nc.partition_id()
        from contextlib import ExitStack
        with ExitStack() as stk:
            p_small = stk.enter_context(tc.tile_pool(name="small", bufs=1))
            p_xw = stk.enter_context(tc.tile_pool(name="xw", bufs=1))
            p_wfold = stk.enter_context(tc.tile_pool(name="wfp", bufs=1))

            sb_qb = p_small.tile([128, NF], F32)
            sb_kb = p_small.tile([128, NF], F32)
            sb_easc = p_small.tile([128, NT], F32)
            sb_ident = p_small.tile([128, 128], BF16)
            sb_wfold = p_wfold.tile([128, NF * E], BF16)

            # one big x DMA on the sync queue (single issue, splits
            # across all DMA engines); weights + small tensors on scalar
            sb_wq = p_xw.tile([128, NE * F], BF16)
            sb_wk = p_xw.tile([128, NE * F], BF16)
            sb_wv = p_xw.tile([128, NE * F], BF16)
            sb_x = p_xw.tile([128, NE * T], BF16, name="xall")
            nc.sync.dma_start(out=sb_x[:], in_=d_x[:])
            nc.scalar.dma_start(out=sb_wk[:], in_=d_wk[:])
            nc.scalar.dma_start(out=sb_wq[:], in_=d_wq[:])
            nc.scalar.dma_start(out=sb_wv[:], in_=d_wv[:])
            nc.scalar.dma_start(out=sb_qb[:], in_=d_qb[:])
            nc.scalar.dma_start(out=sb_kb[:], in_=d_kb[:])
            nc.scalar.dma_start(out=sb_easc[:], in_=d_easc[:])
            nc.scalar.dma_start(out=sb_ident[:], in_=d_ident[:])
            nc.scalar.dma_start(out=sb_wfold[:], in_=d_wfold[:])

            with tc.If(pid < 4) as cmp:
                with ExitStack() as astk:
                    pools = {
                        "psc": astk.enter_context(
                            tc.tile_pool(name="psc", bufs=2, space="PSUM")),
                        "po": astk.enter_context(
                            tc.tile_pool(name="po", bufs=2, space="PSUM")),
                        "ptr": astk.enter_context(
                            tc.tile_pool(name="ptr", bufs=1, space="PSUM")),
                        "pf": astk.enter_context(
                            tc.tile_pool(name="pf", bufs=1, space="PSUM")),
                        "wt": astk.enter_context(tc.tile_pool(name="wt", bufs=2)),
                        "rec": astk.enter_context(tc.tile_pool(name="rec", bufs=2)),
                        "on": astk.enter_context(tc.tile_pool(name="on", bufs=2)),
                        "outT": astk.enter_context(tc.tile_pool(name="outT", bufs=2)),
                        "fin": astk.enter_context(tc.tile_pool(name="fin", bufs=2)),
                        "qt": astk.enter_context(tc.tile_pool(name="qt", bufs=1)),
                        "kt": astk.enter_context(tc.tile_pool(name="kt", bufs=1)),
                        "vp": astk.enter_context(tc.tile_pool(name="vp", bufs=1)),
                    }
                    _branch_regular(nc, tc, pools, sb_x, sb_wq, sb_wk, sb_wv,
                                    sb_qb, sb_kb, sb_easc, sb_ident, sb_wfold,
                                    d_out)
            with cmp.Else():
                with ExitStack() as astk:
                    pools = {
                        "psc": astk.enter_context(
                            tc.tile_pool(name="cpsc", bufs=3, space="PSUM")),
                        "po": astk.enter_context(
                            tc.tile_pool(name="cpo", bufs=2, space="PSUM")),
                        "ptr": astk.enter_context(
                            tc.tile_pool(name="cptr", bufs=1, space="PSUM")),
                        "pf": astk.enter_context(
                            tc.tile_pool(name="cpf", bufs=1, space="PSUM")),
                        "wt": astk.enter_context(tc.tile_pool(name="cwt", bufs=2)),
                        "wtr": astk.enter_context(tc.tile_pool(name="cwtr", bufs=2)),
                        "ecm": astk.enter_context(tc.tile_pool(name="cecm", bufs=2)),
                        "rec": astk.enter_context(tc.tile_pool(name="crec", bufs=2)),
                        "on": astk.enter_context(tc.tile_pool(name="con", bufs=2)),
                        "outT": astk.enter_context(tc.tile_pool(name="coutT", bufs=2)),
                        "fin": astk.enter_context(tc.tile_pool(name="cfin", bufs=2)),
                        "qt": astk.enter_context(tc.tile_pool(name="cqt", bufs=1)),
                        "kt": astk.enter_context(tc.tile_pool(name="ckt", bufs=1)),
                        "vp": astk.enter_context(tc.tile_pool(name="cvp", bufs=1)),
                    }
                    _branch_cultural(nc, tc, pools, sb_x, sb_wq, sb_wk, sb_wv,
                                     sb_qb, sb_kb, sb_easc, sb_ident, sb_wfold,
                                     d_out, d_ecm)
    nc.compile()
    return nc


def _get_nc():
    global _NC_CACHE
    if _NC_CACHE is None:
        _NC_CACHE = _build_nc()
    return _NC_CACHE


def _chunked_T(a):
    """[in,out]-style [768, X] -> [128, 6*X] with e-chunk-major layout."""
    e, x = a.shape
    return np.ascontiguousarray(
        a.reshape(e // 128, 128, x).transpose(1, 0, 2).reshape(128, (e // 128) * x))


def kernel(hidden_states, cultural_mask, attention_mask,
           rq_w, rk_w, rv_w, ro_w, cq_w, ck_w, cv_w, co_w,
           rq_b, rk_b, rv_b, ro_b, cq_b, ck_b, cv_b, co_b,
           r_cb, c_cb, out_w, out_b):
    hidden_states = np.asarray(hidden_states)
    nc = _get_nc()
    Wo1 = np.asarray(out_w[:E], np.float64)
    Wo2 = np.asarray(out_w[E:], np.float64)
    wfold_reg = (np.asarray(ro_w, np.float64) @ Wo1)
    wfold_cul = (np.asarray(co_w, np.float64) @ Wo2)
    r_cb_flat = np.asarray(r_cb, np.float64).reshape(-1)
    c_cb_flat = np.asarray(c_cb, np.float64).reshape(-1)
    qb_reg_full = np.asarray(rq_b, np.float64) + r_cb_flat
    qb_cul_full = np.asarray(cq_b, np.float64) + c_cb_flat

    ident = np.eye(128).astype(ml_dtypes.bfloat16)
    ecm_zero = np.zeros((T, T), ml_dtypes.bfloat16)
    ones_easc = np.ones((128, NT), np.float32)
    in_maps = []
    for core in range(8):
        if core < 4:
            b, h0 = core // 2, (core % 2) * 3
            cols = slice(h0 * 128, h0 * 128 + F)
            wq_l, wk_l, wv_l = rq_w[:, cols], rk_w[:, cols], rv_w[:, cols]
            qb_l = qb_reg_full[cols]
            kb_l = np.asarray(rk_b, np.float64)[cols]
            wfold_l = wfold_reg[cols]
            ecm_l = ecm_zero
            easc_l = np.ascontiguousarray(
                np.exp(np.asarray(attention_mask[b, 0, 0, :], np.float64))
                .astype(np.float32).reshape(NT, 128).T)
        else:
            b, h = (core - 4) // 2, (core - 4) % 2
            cols = slice(h * F, (h + 1) * F)
            wq_l, wk_l, wv_l = cq_w[:, cols], ck_w[:, cols], cv_w[:, cols]
            qb_l = qb_cul_full[cols]
            kb_l = np.asarray(ck_b, np.float64)[cols]
            wfold_l = wfold_cul[cols]
            cm = (np.asarray(cultural_mask[b], np.float64).T
                  + np.asarray(attention_mask[b, 0, 0, :], np.float64)[:, None])
            ecm_l = np.exp(cm).astype(ml_dtypes.bfloat16)
            easc_l = ones_easc
        xT = np.ascontiguousarray(np.asarray(hidden_states[b], np.float32).T)
        im = {
            "wq": _chunked_T(np.asarray(wq_l, np.float32)).astype(ml_dtypes.bfloat16),
            "wk": _chunked_T(np.asarray(wk_l, np.float32)).astype(ml_dtypes.bfloat16),
            "wv": _chunked_T(np.asarray(wv_l, np.float32)).astype(ml_dtypes.bfloat16),
            "qb": np.ascontiguousarray(np.asarray(qb_l, np.float32).reshape(NF, 128).T),
            "kb": np.ascontiguousarray(np.asarray(kb_l, np.float32).reshape(NF, 128).T),
            "easc": easc_l,
            "wfold": _chunked_T(np.asarray(wfold_l, np.float32)).astype(ml_dtypes.bfloat16),
            "ident": ident,
            "ecm": ecm_l,
        }
        im["xall"] = _chunked_T(xT).astype(ml_dtypes.bfloat16)
        in_maps.append(im)

    res = run_bass_kernel_spmd(nc, in_maps, list(range(8))).results

    bias_total = (np.asarray(out_b, np.float64)
                  + np.asarray(ro_b, np.float64) @ Wo1
                  + np.asarray(co_b, np.float64) @ Wo2
                  + np.asarray(rv_b, np.float64) @ np.asarray(ro_w, np.float64) @ Wo1
                  + np.asarray(cv_b, np.float64) @ np.asarray(co_w, np.float64) @ Wo2)
    out = np.empty((B, T, E), np.float32)
    for b in range(B):
        acc = (res[2 * b]["out"].astype(np.float64)
               + res[2 * b + 1]["out"].astype(np.float64)
               + res[4 + 2 * b]["out"].astype(np.float64)
               + res[5 + 2 * b]["out"].astype(np.float64)
               + bias_total)
        out[b] = acc.astype(np.float32)
    return out
